# revision 54
# baseline (speedup 1.0000x reference)
"""Trainium2 Bass kernel: sequence-parallel multi-head self-attention block.

y = proj(softmax(Q K^T / sqrt(D)) V) + b_proj for B=1, N=4096, C=768, H=12,
sharded over 8 NeuronCores by sequence (512 query rows per core).

Design (vs the 412us v1 baseline):
  - the entire phase-2 t-loop runs in ONE PE tiling mode (64x128 row
    tiling): score matmuls contract over D=64 per head (T0 rows 0-63 /
    T8 rows 64-127 concurrent, separate PSUM banks), and the attn@V
    chains are split into keys-lo (T0) / keys-hi (T8) halves that
    accumulate into SEPARATE PSUM banks concurrently (summed at
    finalize). v1 interleaved 64-row score MMs with 128-row chain MMs,
    paying a PE tiling-mode drain on every switch (~4/t); this pays
    zero inside the loop. PSUM: 2 sc bufs (2 banks each) + 4 ob = 8/8.
  - ONE collective per head pair [K tile p | V heads 2p,2p+1 swizzled
    [mt, sub, ch] on key%128 partitions], issued in consumption order.
    Each CC op costs ~11us fixed CC-core overhead plus a ~20us
    completion lag, so 7 small ops (v1) lose ~60us vs 6 aligned ones; a
    tiny warm-up AllGather absorbs the 35-90us (run-variable) CC init
    while the QKV projection computes.
  - kt loads are ONE rearranged DMA descriptor; vt 4 per sub
    (per-descriptor overhead was ~14us on the post-gather critical
    path in v1).
  - exp per WHOLE score tile [128, 1024]: ACT takes 21/32 tiles (real
    exp), DVE 11/32 (Schraudolph int16-bitcast fast-exp) -- DVE pays a
    pipe-drain roughly equal to op duration, so its effective tile cost
    is ~2.2us vs ACT ~1.15us. The last two tiles split across both
    engines (the next pair's first scores wait on those sc banks).
  - output projection: all 8 (mt, half) groups hold open accumulations
    in 8 PSUM banks and run their k=0..4 terms first, overlapping the
    last pair's finalize chain; only the k=5 term (aoTn[5]) + bias land
    after it. A naive group-serial order stalls the PE FIFO ~7us.
  - finalize per pair: PSUM->SBUF drains of both chain halves split
    across DVE (sub 0) and ACT (sub 1) so all 4 ob banks free fast for
    the next pair's chains; Z = ones-row 64 of lo+hi, normalize via
    reciprocal_approx_fast + Pool partition_broadcast + DVE mult. The
    adds/mult must NOT go on gpsimd: its tensor_tensor lives in the
    'standard' Q7 ucode library, partition_broadcast in 'attn', and
    mixing them reloads the library every pair (~10us) -- an explicit
    load_library(proxy), which contains both, guards against that. The
    vt ones-memset runs on DVE, not Pool, so it never queues behind
    finalize arithmetic.

Measured on 8 trn2 NeuronCores: total ~= 300us + the CC-init draw
(16.7-119us observed across identical binaries): best 316.8us
(16.7us draw), typical 340-350us (~40us draws). HAM/GPIO power-state
drift adds further run noise. rel err ~1.0-1.4e-2 run-variable vs the
fp32 reference (scale-relative absmax). Steady state ~33.4us/head-
pair: ACT/DVE exp wall and PE matmul slots are within ~10% of each
other; startup is the collective floor (CC boot 21 + init draw +
first gather + ~20us completion lag).
"""

import numpy as np

CORES = 8
N = 4096
S = N // CORES          # 512 query rows per core
C = 768
H = 12
D = 64
HP = H // 2             # 6 head-pair tiles
CT = C // 128           # 6 contraction tiles over C
KT = N // 128           # 32 key tiles
MT = S // 128           # 4 local seq tiles
SCALE = float(D) ** -0.5
EXP_A = SCALE * float(np.log2(np.e)) * 128.0
EXP_MAGIC = 16256.0 - 7.42

_COMPILED = None


def _build():
    from contextlib import ExitStack

    import concourse.tile as tile
    from concourse import bacc, mybir

    import ml_dtypes

    f32 = mybir.dt.float32
    f32r = mybir.dt.float32r
    bf16 = mybir.dt.bfloat16
    i16 = mybir.dt.int16
    EXP = mybir.ActivationFunctionType.Exp
    COPY = mybir.ActivationFunctionType.Copy
    MULT = mybir.AluOpType.mult

    nc = bacc.Bacc("TRN2", target_bir_lowering=False, debug=False,
                   num_devices=CORES)

    xT = nc.dram_tensor("xT", [C, S], f32, kind="ExternalInput")
    w_qkv = nc.dram_tensor("w_qkv", [C, 3 * C], f32, kind="ExternalInput")
    w_proj = nc.dram_tensor("w_proj", [C, C], f32, kind="ExternalInput")
    b_proj = nc.dram_tensor("b_proj", [1, C], f32, kind="ExternalInput")
    y = nc.dram_tensor("y", [S, C], f32, kind="ExternalOutput")

    # per-pair bounce/gather buffers [128, 1024] bf16: cols 0:512 = K tile
    # p; cols 512:1024 = V heads 2p/2p+1 swizzled [mt(4), sub(2), ch(64)].
    # One collective per pair, issued in consumption order, so arrivals
    # (~24us apart incl. ~11us/op CC overhead) stay ahead of the ~34us/pair
    # compute. V swizzle keys on key%128 partitions like K.
    bnc_g = [nc.dram_tensor(f"bnc_g{i}", [128, 1024], bf16)
             for i in range(HP)]
    gat_g = [nc.dram_tensor(f"gat_g{i}", [CORES * 128, 1024], bf16,
                            addr_space="Shared")
             for i in range(HP)]

    groups = [list(range(CORES))]

    def allgather(src, dst):
        nc.gpsimd.collective_compute(
            "AllGather", mybir.AluOpType.bypass, replica_groups=groups,
            ins=[src.ap()], outs=[dst.ap()])

    from concourse import library_config

    with tile.TileContext(nc) as tc, ExitStack() as ctx:
        # load the one Q7 library that has BOTH tensor_tensor and
        # partition_broadcast, so Pool can run the finalize arithmetic
        # without per-pair library reloads (standard vs attn flip-flop)
        nc.gpsimd.load_library(library_config.proxy)
        const_pool = ctx.enter_context(tc.tile_pool(name="const", bufs=1))
        qT_pool = ctx.enter_context(tc.tile_pool(name="qT", bufs=1))
        aoT_pool = ctx.enter_context(tc.tile_pool(name="aoT", bufs=1))
        wp_pool = ctx.enter_context(tc.tile_pool(name="wp", bufs=1))

        # warm-up collective: absorbs the ~45-90us first-collective init
        # while phase 1 computes
        wup_in = nc.dram_tensor("wup_in", [1, 16], bf16)
        wup_out = nc.dram_tensor("wup_out", [CORES, 16], bf16,
                                 addr_space="Shared")
        allgather(wup_in, wup_out)

        ones_dram_bf = nc.inline_tensor(
            np.ones((1, 128), ml_dtypes.bfloat16), name="ones_dram_bf")
        ones_bf = const_pool.tile([1, 128], bf16, name="ones_bf")
        nc.sync.dma_start(ones_bf[:], ones_dram_bf[:, :])
        bp_sb = const_pool.tile([1, C], f32, name="bp_sb")
        nc.sync.dma_start(bp_sb[:], b_proj[:, :])
        bpb_sb = const_pool.tile([1, C], bf16, name="bpb_sb")

        qT_sb = [qT_pool.tile([128, S], bf16, name=f"qT{m}") for m in range(CT)]
        aoTn = [aoT_pool.tile([128, S], bf16, name=f"aoTn{m}")
                for m in range(CT)]
        wpb_sb = [wp_pool.tile([128, C], bf16, name=f"wpb{k}")
                  for k in range(CT)]

        # ---- phase 1: qkv projection + split allgathers ----
        with tc.tile_pool(name="xw", bufs=1) as xw_pool, \
             tc.tile_pool(name="st1", bufs=1) as st1_pool, \
             tc.tile_pool(name="ps1", bufs=1, space="PSUM") as ps1_pool:
            xT_sb = [xw_pool.tile([128, S], f32r, name=f"xTs{k}")
                     for k in range(CT)]
            wqK0 = [xw_pool.tile([128, 128], f32r, name=f"wqK0_{k}")
                    for k in range(CT)]
            wqK = [xw_pool.tile([128, C], f32r, name=f"wqK{k}")
                   for k in range(CT)]
            wqVa = [xw_pool.tile([128, C // 2], f32r, name=f"wqVa{k}")
                    for k in range(CT)]
            wqVb = [xw_pool.tile([128, C // 2], f32r, name=f"wqVb{k}")
                    for k in range(CT)]
            wqQ = [xw_pool.tile([128, C], f32r, name=f"wqQ{k}")
                   for k in range(CT)]
            # DMA priority order: x, the K-tile-0 weight slice, then the
            # first-half V weights -- K0 and V heads 0-5 feed the first
            # gathers, which gate phase 2 on lucky CC-init draws
            for k in range(CT):
                nc.sync.dma_start(xT_sb[k][:],
                                  xT[128 * k:128 * (k + 1), :].bitcast(f32r))
            for k in range(CT):
                nc.sync.dma_start(
                    wqK0[k][:], w_qkv[128 * k:128 * (k + 1), C:C + 128]
                    .bitcast(f32r))
            for k in range(CT):
                nc.sync.dma_start(
                    wqVa[k][:], w_qkv[128 * k:128 * (k + 1),
                                      2 * C:2 * C + C // 2].bitcast(f32r))
            for k in range(CT):
                nc.sync.dma_start(
                    wqK[k][:], w_qkv[128 * k:128 * (k + 1), C:2 * C]
                    .bitcast(f32r))
            for k in range(CT):
                nc.sync.dma_start(
                    wqQ[k][:], w_qkv[128 * k:128 * (k + 1), 0:C]
                    .bitcast(f32r))
            for k in range(CT):
                nc.sync.dma_start(
                    wqVb[k][:], w_qkv[128 * k:128 * (k + 1),
                                      2 * C + C // 2:3 * C].bitcast(f32r))

            copy_flip = [0]

            def psum_copy(dst, src):
                # alternate ACT / DVE for PSUM->SBUF drains
                if copy_flip[0] % 2 == 0:
                    nc.vector.tensor_copy(dst, src)
                else:
                    nc.scalar.activation(dst, src, COPY)
                copy_flip[0] += 1

            def kq_tile(w_tiles, i, dst):
                ps = ps1_pool.tile([128, S], f32, name="ps_kq",
                                   tag="ps_kq", bufs=3)
                for k in range(CT):
                    nc.tensor.matmul(ps[:],
                                     w_tiles[k][:, 128 * i:128 * (i + 1)],
                                     xT_sb[k][:],
                                     start=(k == 0), stop=(k == CT - 1))
                psum_copy(dst[:], ps[:])

            def v_tiles(dsts, n0):
                # dsts: list of (bnc, colbase, c0, c1); the [S, C//2] half
                # is computed per 128-row seq tile mt and stored swizzled
                # as bnc[key%128, colbase + mt*(c1-c0) + ch]
                for mt in range(MT):
                    vst = st1_pool.tile([128, C // 2], bf16, name="vst",
                                        tag="vst", bufs=3)
                    ps = ps1_pool.tile([128, C // 2], f32, name="ps_v",
                                       tag="ps_v", bufs=3)
                    wqVh = wqVa if n0 == 0 else wqVb
                    for k in range(CT):
                        nc.tensor.matmul(
                            ps[:],
                            xT_sb[k][:, 128 * mt:128 * (mt + 1)],
                            wqVh[k][:],
                            start=(k == 0), stop=(k == CT - 1))
                    psum_copy(vst[:], ps[:])
                    for (bnc, cb, c0, c1) in dsts:
                        w = c1 - c0
                        nc.sync.dma_start(
                            bnc[:, cb + mt * w:cb + (mt + 1) * w],
                            vst[:, c0:c1])

            def k_tile_store(i, bnc, col0, w_tiles=None):
                kst = st1_pool.tile([128, S], bf16, name="kst",
                                    tag="kst", bufs=3)
                kq_tile(w_tiles or wqK, i, kst)
                nc.sync.dma_start(bnc[:, col0:col0 + S], kst[:])

            # pair-0's K tile (from the prioritized weight slice), then the
            # V first half distributed into the pair-0/1/2 buffers, then one
            # gather per pair in consumption order
            k_tile_store(0, bnc_g[0], 0, wqK0)
            v_tiles([(bnc_g[0], 512, 0, 128), (bnc_g[1], 512, 128, 256),
                     (bnc_g[2], 512, 256, 384)], 0)
            allgather(bnc_g[0], gat_g[0])
            k_tile_store(1, bnc_g[1], 0)
            allgather(bnc_g[1], gat_g[1])
            k_tile_store(2, bnc_g[2], 0)
            allgather(bnc_g[2], gat_g[2])
            # Q tiles 0-2 (needed by first pairs)
            for i in range(3):
                kq_tile(wqQ, i, qT_sb[i])
            # second half: K3 + V heads 6-11, then per-pair gathers
            k_tile_store(3, bnc_g[3], 0)
            v_tiles([(bnc_g[3], 512, 0, 128), (bnc_g[4], 512, 128, 256),
                     (bnc_g[5], 512, 256, 384)], C // 2)
            allgather(bnc_g[3], gat_g[3])
            k_tile_store(4, bnc_g[4], 0)
            allgather(bnc_g[4], gat_g[4])
            k_tile_store(5, bnc_g[5], 0)
            allgather(bnc_g[5], gat_g[5])
            # Q tiles 3-5
            for i in range(3, CT):
                kq_tile(wqQ, i, qT_sb[i])

            # w_proj load + bf16 conversion on Pool (idle engine)
            wp_sb = [xw_pool.tile([128, C], f32, name=f"wp{k}")
                     for k in range(CT)]
            for k in range(CT):
                nc.sync.dma_start(wp_sb[k][:],
                                  w_proj[128 * k:128 * (k + 1), :])
            for k in range(CT):
                nc.gpsimd.tensor_copy(wpb_sb[k][:], wp_sb[k][:])
            nc.gpsimd.tensor_copy(bpb_sb[:], bp_sb[:])

        # ---- phase 2: attention ----
        # Single PE tiling mode (64x128) throughout: score MMs contract
        # D=64 per head on T0/T8; chain MMs contract keys split lo/hi on
        # T0/T8 into separate PSUM banks (ob_lo / ob_hi), summed at
        # finalize. Z rides along as the ones-row 64 of both chains.
        with tc.tile_pool(name="kt", bufs=1) as kt_pool, \
             tc.tile_pool(name="vt", bufs=1) as vt_pool, \
             tc.tile_pool(name="pt", bufs=1) as pt_pool, \
             tc.tile_pool(name="fin", bufs=1) as fin_pool, \
             tc.tile_pool(name="sc", bufs=1, space="PSUM") as sc_pool, \
             tc.tile_pool(name="ob", bufs=1, space="PSUM") as ob_pool:

            def finalize_pair(hp, ob_lo, ob_hi):
                # ACT (only engine with spare PSUM-read capacity) drains
                # both chain halves to SBUF fast, freeing the ob banks for
                # the next pair's chains; Pool does the adds/normalize
                # multiply; DVE only the reciprocal.
                for sub in range(2):
                    # sub 0 drains on DVE, sub 1 on ACT -> both subs' ob
                    # banks free in parallel for the next pair's chains
                    hi_s = fin_pool.tile([65, S], f32, name="hi_s",
                                         tag=f"hi_s{sub}", bufs=2)
                    lo_s = fin_pool.tile([65, S], f32, name="lo_s",
                                         tag=f"lo_s{sub}", bufs=2)
                    if sub == 0:
                        nc.vector.tensor_copy(hi_s[:], ob_hi[sub][0:65, :])
                        nc.vector.tensor_copy(lo_s[:], ob_lo[sub][0:65, :])
                    else:
                        nc.scalar.activation(hi_s[:], ob_hi[sub][0:65, :],
                                             COPY)
                        nc.scalar.activation(lo_s[:], ob_lo[sub][0:65, :],
                                             COPY)
                    # NOTE: the adds/mul must NOT go on gpsimd — its
                    # tensor_tensor lives in the 'standard' Q7 library while
                    # partition_broadcast lives in 'attn', and mixing them
                    # forces a library reload every pair
                    zt = fin_pool.tile([1, S], f32, name="zt",
                                       tag=f"zt{sub}", bufs=2)
                    nc.vector.tensor_add(zt[:], lo_s[64:65, :],
                                         hi_s[64:65, :])
                    rzv = fin_pool.tile([1, S], f32, name="rzv",
                                        tag=f"rzv{sub}", bufs=2)
                    nc.vector.reciprocal_approx_fast(rzv[:], zt[:])
                    rzb = fin_pool.tile([64, S], f32, name="rzb",
                                        tag=f"rzb{sub}", bufs=2)
                    nc.gpsimd.partition_broadcast(rzb[:], rzv[:])
                    tmp = fin_pool.tile([64, S], f32, name="tmp",
                                        tag=f"tmp{sub}", bufs=2)
                    nc.vector.tensor_add(tmp[:], lo_s[0:64, :],
                                         hi_s[0:64, :])
                    aon = fin_pool.tile([64, S], bf16, name="aon",
                                        tag=f"aon{sub}", bufs=2)
                    nc.vector.tensor_mul(aon[:], rzb[:], tmp[:])
                    nc.sync.dma_start(aoTn[hp][64 * sub:64 * (sub + 1), :],
                                      aon[:])

            for hp in range(HP):
                kt = kt_pool.tile([128, N], bf16, name="kt", tag="kt", bufs=2)
                # two DMAs (first/second core-halves): pair-0's scores for
                # key tiles 0-15 start ~3.5us sooner after the gather lands.
                # The vt loads are issued BETWEEN the halves: chains need vt
                # by t~3-10 while scores need the second kt half only at
                # t=16, so this ordering removes a ~3.5us chain stall.
                src = gat_g[hp].rearrange("(r p) s -> p r s", p=128)
                kt_v = kt[:].rearrange("p (r s) -> p r s", r=CORES)
                nc.sync.dma_start(kt_v[:, 0:CORES // 2, :],
                                  src[:, 0:CORES // 2, 0:S])

                vts = []
                for sub in range(2):
                    vt = vt_pool.tile([128, KT * 65], bf16, name=f"vt{sub}",
                                      tag=f"vt{sub}", bufs=2)
                    vt_v = vt[:].rearrange("p (t c) -> p t c", c=65)
                    # DVE, not Pool: Pool's FIFO carries the previous
                    # pair's finalize arithmetic and would delay this
                    nc.vector.memset(vt_v[:, :, D], 1.0)
                    # 4 DMAs (one per mt): vt[p, 4r+mt, 0:D] <-
                    # gat[128r+p, 512 + mt*128 + sub*64 : +D]
                    vt_v4 = vt[:].rearrange("p (r t c) -> p r t c",
                                            r=CORES, c=65)
                    for mt in range(MT):
                        cc0 = 512 + mt * 128 + sub * D
                        nc.sync.dma_start(vt_v4[:, :, mt, 0:D],
                                          src[:, :, cc0:cc0 + D])
                    vts.append(vt)
                nc.sync.dma_start(kt_v[:, CORES // 2:CORES, :],
                                  src[:, CORES // 2:CORES, 0:S])

                ob_lo = [ob_pool.tile([128, S], f32, name=f"obl{sub}",
                                      tag=f"obl{sub}", bufs=1)
                         for sub in range(2)]
                ob_hi = [ob_pool.tile([128, S], f32, name=f"obh{sub}",
                                      tag=f"obh{sub}", bufs=1)
                         for sub in range(2)]

                # chains trail their own pair's scores by a few steps (vt
                # DMA margin; V arrives in the same gather as K)
                # delay 3: at delay 2 the chains catch pt(j) before its
                # exp completes (~2.2us stall per pair in the trace)
                delay, rate = (10, 2) if hp == 0 else (3, 1)
                done = [0, 0]

                def chain_step(sub, j):
                    nc.tensor.matmul(
                        ob_lo[sub][0:65, :],
                        vts[sub][0:64, 65 * j:65 * (j + 1)],
                        pts[j][0:64, S * sub:S * (sub + 1)],
                        start=(j == 0), stop=(j == KT - 1))
                    nc.tensor.matmul(
                        ob_hi[sub][0:65, :],
                        vts[sub][64:128, 65 * j:65 * (j + 1)],
                        pts[j][64:128, S * sub:S * (sub + 1)],
                        start=(j == 0), stop=(j == KT - 1))

                def run_chains(t, sub):
                    quota = min(KT, len(pts), max(0, t - delay + 1) * rate)
                    while done[sub] < quota:
                        chain_step(sub, done[sub])
                        done[sub] += 1

                pts = []
                for t in range(KT + 2):
                    if t < KT:
                        sc = sc_pool.tile([128, 2 * S], f32, name="sc",
                                          tag="sc", bufs=2)
                        nc.tensor.matmul(
                            sc[:, 0:S], kt[0:64, 128 * t:128 * (t + 1)],
                            qT_sb[hp][0:64, :], start=True, stop=True)
                        nc.tensor.matmul(
                            sc[:, S:2 * S], kt[64:128, 128 * t:128 * (t + 1)],
                            qT_sb[hp][64:128, :], start=True, stop=True)
                        pt = pt_pool.tile([128, 2 * S], bf16, name="pt",
                                          tag="pt", bufs=12)
                        # DVE pays a pipe-drain ~= op cost on its big exp
                        # ops (effective ~2.2us/tile vs ACT ~1.15us), so
                        # ACT takes 21 of 32 tiles. The last two tiles are
                        # split across both engines: the next pair's first
                        # scores wait on these sc banks, so clearing the
                        # exp backlog fast shrinks the pair-boundary gap.
                        if t >= KT - 2:
                            nc.scalar.activation(pt[:, 0:S], sc[:, 0:S],
                                                 EXP, scale=SCALE)
                            nc.vector.tensor_scalar(
                                pt[:, S:2 * S].bitcast(i16), sc[:, S:2 * S],
                                EXP_A, EXP_MAGIC, MULT, mybir.AluOpType.add)
                        elif (t * 21) % 32 < 21:
                            nc.scalar.activation(pt[:], sc[:], EXP,
                                                 scale=SCALE)
                        else:
                            nc.vector.tensor_scalar(
                                pt[:].bitcast(i16), sc[:], EXP_A, EXP_MAGIC,
                                MULT, mybir.AluOpType.add)
                        pts.append(pt)
                        run_chains(t - 2, 0)
                        run_chains(t - 2, 1)
                    else:
                        run_chains(KT + delay, 0)
                        run_chains(KT + delay, 1)
                finalize_pair(hp, ob_lo, ob_hi)

        # ---- phase 3: output projection + bias ----
        # All 8 (mt, half) groups hold an open accumulation in their own
        # PSUM bank and run k=0..4 first: those 40 MMs overlap the last
        # pair's finalize chain (which produces aoTn[5], the k=5 input).
        # A k=5-early ordering would stall the whole PE FIFO on group 0.
        with tc.tile_pool(name="yst", bufs=1) as y_pool, \
             tc.tile_pool(name="fo", bufs=1, space="PSUM") as fo_pool:
            ysts = [y_pool.tile([128, C], f32, name=f"yst{mt}",
                                tag="yst", bufs=4) for mt in range(MT)]
            halves = ((0, 384), (384, 768))
            fos = {}
            for mt in range(MT):
                for half, (n0, n1) in enumerate(halves):
                    fo = fo_pool.tile([128, 384], f32, name="fo", tag="fo",
                                      bufs=8)
                    for k in range(CT - 1):
                        nc.tensor.matmul(
                            fo[:],
                            aoTn[k][:, 128 * mt:128 * (mt + 1)],
                            wpb_sb[k][:, n0:n1],
                            start=(k == 0), stop=False)
                    fos[(mt, half)] = fo
            for mt in range(MT):
                for half, (n0, n1) in enumerate(halves):
                    fo = fos[(mt, half)]
                    nc.tensor.matmul(
                        fo[:],
                        aoTn[CT - 1][:, 128 * mt:128 * (mt + 1)],
                        wpb_sb[CT - 1][:, n0:n1],
                        start=False, stop=False)
                    nc.tensor.matmul(fo[:], ones_bf[0:1, :],
                                     bpb_sb[0:1, n0:n1],
                                     start=False, stop=True)
                    if (2 * mt + half) % 2 == 0:
                        nc.vector.tensor_copy(ysts[mt][:, n0:n1], fo[:])
                    else:
                        nc.scalar.activation(ysts[mt][:, n0:n1], fo[:], COPY)
                nc.sync.dma_start(y[128 * mt:128 * (mt + 1), :], ysts[mt][:])

    nc.compile()
    return nc


def _get_compiled():
    global _COMPILED
    if _COMPILED is None:
        _COMPILED = _build()
    return _COMPILED


def _run(inputs, trace=False):
    from concourse.bass_utils import run_bass_kernel_spmd

    nc = _get_compiled()
    x = np.asarray(inputs["x"], dtype=np.float32)
    w_qkv = np.ascontiguousarray(np.asarray(inputs["w_qkv"], dtype=np.float32))
    w_proj = np.ascontiguousarray(np.asarray(inputs["w_proj"], dtype=np.float32))
    b_proj = np.ascontiguousarray(
        np.asarray(inputs["b_proj"], dtype=np.float32).reshape(1, C))
    xT_full = np.ascontiguousarray(x[0].T)  # [C, N]

    in_maps = []
    for c in range(CORES):
        in_maps.append({
            "xT": np.ascontiguousarray(xT_full[:, S * c:S * (c + 1)]),
            "w_qkv": w_qkv,
            "w_proj": w_proj,
            "b_proj": b_proj,
        })
    res = run_bass_kernel_spmd(nc, in_maps, core_ids=list(range(CORES)),
                               trace=trace)
    out = np.concatenate([res.results[c]["y"] for c in range(CORES)], axis=0)
    return out[None, :, :].astype(np.float32), res


def _spot_check(out, inputs, rows=(17, 1031, 2063, 3571)) -> bool:
    """Cheap numpy verification of a few output rows (guards against a rare
    intermittent device-side corruption; full fp32 math on 4 rows)."""
    x = np.asarray(inputs["x"], dtype=np.float32)[0]
    w_qkv = np.asarray(inputs["w_qkv"], dtype=np.float32)
    w_proj = np.asarray(inputs["w_proj"], dtype=np.float32)
    b_proj = np.asarray(inputs["b_proj"], dtype=np.float32).reshape(-1)
    kf = x @ w_qkv[:, C:2 * C]
    vf = x @ w_qkv[:, 2 * C:3 * C]
    scale = np.abs(out).max() + 1e-30
    for r in rows:
        q = x[r] @ w_qkv[:, 0:C]
        ao = np.empty(C, np.float32)
        for h in range(H):
            sc = kf[:, D * h:D * (h + 1)] @ q[D * h:D * (h + 1)] * SCALE
            p = np.exp(sc - sc.max())
            ao[D * h:D * (h + 1)] = (p @ vf[:, D * h:D * (h + 1)]) / p.sum()
        yr = ao @ w_proj + b_proj
        if np.abs(out[0, r] - yr).max() / scale > 5e-2:
            return False
    return True


def kernel(**inputs) -> np.ndarray:
    out, _ = _run(inputs, trace=False)
    for _retry in range(2):
        if _spot_check(out, inputs):
            break
        out, _ = _run(inputs, trace=False)
    return out


# revision 55
# speedup vs baseline: 1.3106x; 1.3106x over previous
"""Trainium2 Bass kernel: sequence-parallel multi-head self-attention block.

y = proj(softmax(Q K^T / sqrt(D)) V) + b_proj for B=1, N=4096, C=768, H=12,
sharded over 8 NeuronCores by sequence (512 query rows per core).

Design (vs the 412us v1 baseline):
  - the entire phase-2 t-loop runs in ONE PE tiling mode (64x128 row
    tiling): score matmuls contract over D=64 per head (T0 rows 0-63 /
    T8 rows 64-127 concurrent, separate PSUM banks), and the attn@V
    chains are split into keys-lo (T0) / keys-hi (T8) halves that
    accumulate into SEPARATE PSUM banks concurrently (summed at
    finalize). v1 interleaved 64-row score MMs with 128-row chain MMs,
    paying a PE tiling-mode drain on every switch (~4/t); this pays
    zero inside the loop. PSUM: 2 sc bufs (2 banks each) + 4 ob = 8/8.
  - ONE collective per head pair [K tile p | V heads 2p,2p+1 swizzled
    [mt, sub, ch] on key%128 partitions], issued in consumption order.
    Each CC op costs ~11us fixed CC-core overhead plus a ~20us
    completion lag, so 7 small ops (v1) lose ~60us vs 6 aligned ones; a
    tiny warm-up AllGather absorbs the 35-90us (run-variable) CC init
    while the QKV projection computes.
  - kt loads are ONE rearranged DMA descriptor; vt 4 per sub
    (per-descriptor overhead was ~14us on the post-gather critical
    path in v1).
  - exp per WHOLE score tile [128, 1024]: ACT takes 21/32 tiles (real
    exp), DVE 11/32 (Schraudolph int16-bitcast fast-exp) -- DVE pays a
    pipe-drain roughly equal to op duration, so its effective tile cost
    is ~2.2us vs ACT ~1.15us. The last two tiles split across both
    engines (the next pair's first scores wait on those sc banks).
  - output projection: all 8 (mt, half) groups hold open accumulations
    in 8 PSUM banks and run their k=0..4 terms first, overlapping the
    last pair's finalize chain; only the k=5 term (aoTn[5]) + bias land
    after it. A naive group-serial order stalls the PE FIFO ~7us.
  - finalize per pair: PSUM->SBUF drains of both chain halves split
    across DVE (sub 0) and ACT (sub 1) so all 4 ob banks free fast for
    the next pair's chains; Z = ones-row 64 of lo+hi, normalize via
    reciprocal_approx_fast + Pool partition_broadcast + DVE mult. The
    adds/mult must NOT go on gpsimd: its tensor_tensor lives in the
    'standard' Q7 ucode library, partition_broadcast in 'attn', and
    mixing them reloads the library every pair (~10us) -- an explicit
    load_library(proxy), which contains both, guards against that. The
    vt ones-memset runs on DVE, not Pool, so it never queues behind
    finalize arithmetic.

Measured on 8 trn2 NeuronCores: total ~= 300us + the CC-init draw
(16.7-119us observed across identical binaries): best 316.8us
(16.7us draw), typical 340-350us (~40us draws). HAM/GPIO power-state
drift adds further run noise. rel err ~1.0-1.4e-2 run-variable vs the
fp32 reference (scale-relative absmax). Steady state ~33.4us/head-
pair: ACT/DVE exp wall and PE matmul slots are within ~10% of each
other; startup is the collective floor (CC boot 21 + init draw +
first gather + ~20us completion lag).
"""

import numpy as np

CORES = 8
N = 4096
S = N // CORES          # 512 query rows per core
C = 768
H = 12
D = 64
HP = H // 2             # 6 head-pair tiles
CT = C // 128           # 6 contraction tiles over C
KT = N // 128           # 32 key tiles
MT = S // 128           # 4 local seq tiles
SCALE = float(D) ** -0.5
EXP_A = SCALE * float(np.log2(np.e)) * 128.0
EXP_MAGIC = 16256.0 - 7.42

_COMPILED = None


def _build():
    from contextlib import ExitStack

    import concourse.tile as tile
    from concourse import bacc, mybir

    import ml_dtypes

    f32 = mybir.dt.float32
    f32r = mybir.dt.float32r
    bf16 = mybir.dt.bfloat16
    i16 = mybir.dt.int16
    EXP = mybir.ActivationFunctionType.Exp
    COPY = mybir.ActivationFunctionType.Copy
    MULT = mybir.AluOpType.mult

    nc = bacc.Bacc("TRN2", target_bir_lowering=False, debug=False,
                   num_devices=CORES)

    xT = nc.dram_tensor("xT", [C, S], f32, kind="ExternalInput")
    w_qkv = nc.dram_tensor("w_qkv", [C, 3 * C], f32, kind="ExternalInput")
    w_proj = nc.dram_tensor("w_proj", [C, C], f32, kind="ExternalInput")
    b_proj = nc.dram_tensor("b_proj", [1, C], f32, kind="ExternalInput")
    y = nc.dram_tensor("y", [S, C], f32, kind="ExternalOutput")

    # per-pair bounce/gather buffers [128, 1024] bf16: cols 0:512 = K tile
    # p; cols 512:1024 = V heads 2p/2p+1 swizzled [mt(4), sub(2), ch(64)].
    # One collective per pair, issued in consumption order, so arrivals
    # (~24us apart incl. ~11us/op CC overhead) stay ahead of the ~34us/pair
    # compute. V swizzle keys on key%128 partitions like K.
    bnc_g = [nc.dram_tensor(f"bnc_g{i}", [128, 1024], bf16)
             for i in range(HP)]
    gat_g = [nc.dram_tensor(f"gat_g{i}", [CORES * 128, 1024], bf16,
                            addr_space="Shared")
             for i in range(HP)]

    groups = [list(range(CORES))]

    def allgather(src, dst):
        nc.gpsimd.collective_compute(
            "AllGather", mybir.AluOpType.bypass, replica_groups=groups,
            ins=[src.ap()], outs=[dst.ap()])

    from concourse import library_config

    with tile.TileContext(nc) as tc, ExitStack() as ctx:
        # load the one Q7 library that has BOTH tensor_tensor and
        # partition_broadcast, so Pool can run the finalize arithmetic
        # without per-pair library reloads (standard vs attn flip-flop)
        nc.gpsimd.load_library(library_config.proxy)
        const_pool = ctx.enter_context(tc.tile_pool(name="const", bufs=1))
        qT_pool = ctx.enter_context(tc.tile_pool(name="qT", bufs=1))
        aoT_pool = ctx.enter_context(tc.tile_pool(name="aoT", bufs=1))
        wp_pool = ctx.enter_context(tc.tile_pool(name="wp", bufs=1))

        # warm-up collective: absorbs the ~45-90us first-collective init
        # while phase 1 computes
        wup_in = nc.dram_tensor("wup_in", [1, 16], bf16)
        wup_out = nc.dram_tensor("wup_out", [CORES, 16], bf16,
                                 addr_space="Shared")
        allgather(wup_in, wup_out)

        ones_dram_bf = nc.inline_tensor(
            np.ones((1, 128), ml_dtypes.bfloat16), name="ones_dram_bf")
        ones_bf = const_pool.tile([1, 128], bf16, name="ones_bf")
        nc.sync.dma_start(ones_bf[:], ones_dram_bf[:, :])
        bp_sb = const_pool.tile([1, C], f32, name="bp_sb")
        nc.sync.dma_start(bp_sb[:], b_proj[:, :])
        bpb_sb = const_pool.tile([1, C], bf16, name="bpb_sb")

        qT_sb = [qT_pool.tile([128, S], bf16, name=f"qT{m}") for m in range(CT)]
        aoTn = [aoT_pool.tile([128, S], bf16, name=f"aoTn{m}")
                for m in range(CT)]
        wpb_sb = [wp_pool.tile([128, C], bf16, name=f"wpb{k}")
                  for k in range(CT)]

        # ---- phase 1: qkv projection + split allgathers ----
        with tc.tile_pool(name="xw", bufs=1) as xw_pool, \
             tc.tile_pool(name="st1", bufs=1) as st1_pool, \
             tc.tile_pool(name="ps1", bufs=1, space="PSUM") as ps1_pool:
            xT_sb = [xw_pool.tile([128, S], f32r, name=f"xTs{k}")
                     for k in range(CT)]
            wqK0 = [xw_pool.tile([128, 128], f32r, name=f"wqK0_{k}")
                    for k in range(CT)]
            wqK = [xw_pool.tile([128, C], f32r, name=f"wqK{k}")
                   for k in range(CT)]
            wqVa = [xw_pool.tile([128, C // 2], f32r, name=f"wqVa{k}")
                    for k in range(CT)]
            wqVb = [xw_pool.tile([128, C // 2], f32r, name=f"wqVb{k}")
                    for k in range(CT)]
            wqQ = [xw_pool.tile([128, C], f32r, name=f"wqQ{k}")
                   for k in range(CT)]
            # DMA priority order: x, the K-tile-0 weight slice, then the
            # first-half V weights -- K0 and V heads 0-5 feed the first
            # gathers, which gate phase 2 on lucky CC-init draws
            for k in range(CT):
                nc.sync.dma_start(xT_sb[k][:],
                                  xT[128 * k:128 * (k + 1), :].bitcast(f32r))
            for k in range(CT):
                nc.sync.dma_start(
                    wqK0[k][:], w_qkv[128 * k:128 * (k + 1), C:C + 128]
                    .bitcast(f32r))
            for k in range(CT):
                nc.sync.dma_start(
                    wqVa[k][:], w_qkv[128 * k:128 * (k + 1),
                                      2 * C:2 * C + C // 2].bitcast(f32r))
            for k in range(CT):
                nc.sync.dma_start(
                    wqK[k][:], w_qkv[128 * k:128 * (k + 1), C:2 * C]
                    .bitcast(f32r))
            for k in range(CT):
                nc.sync.dma_start(
                    wqQ[k][:], w_qkv[128 * k:128 * (k + 1), 0:C]
                    .bitcast(f32r))
            for k in range(CT):
                nc.sync.dma_start(
                    wqVb[k][:], w_qkv[128 * k:128 * (k + 1),
                                      2 * C + C // 2:3 * C].bitcast(f32r))

            copy_flip = [0]

            def psum_copy(dst, src):
                # alternate ACT / DVE for PSUM->SBUF drains
                if copy_flip[0] % 2 == 0:
                    nc.vector.tensor_copy(dst, src)
                else:
                    nc.scalar.activation(dst, src, COPY)
                copy_flip[0] += 1

            def kq_tile(w_tiles, i, dst):
                ps = ps1_pool.tile([128, S], f32, name="ps_kq",
                                   tag="ps_kq", bufs=3)
                for k in range(CT):
                    nc.tensor.matmul(ps[:],
                                     w_tiles[k][:, 128 * i:128 * (i + 1)],
                                     xT_sb[k][:],
                                     start=(k == 0), stop=(k == CT - 1))
                psum_copy(dst[:], ps[:])

            def v_tiles(dsts, n0):
                # dsts: list of (bnc, colbase, c0, c1); the [S, C//2] half
                # is computed per 128-row seq tile mt and stored swizzled
                # as bnc[key%128, colbase + mt*(c1-c0) + ch]
                for mt in range(MT):
                    vst = st1_pool.tile([128, C // 2], bf16, name="vst",
                                        tag="vst", bufs=3)
                    ps = ps1_pool.tile([128, C // 2], f32, name="ps_v",
                                       tag="ps_v", bufs=3)
                    wqVh = wqVa if n0 == 0 else wqVb
                    for k in range(CT):
                        nc.tensor.matmul(
                            ps[:],
                            xT_sb[k][:, 128 * mt:128 * (mt + 1)],
                            wqVh[k][:],
                            start=(k == 0), stop=(k == CT - 1))
                    psum_copy(vst[:], ps[:])
                    for (bnc, cb, c0, c1) in dsts:
                        w = c1 - c0
                        nc.sync.dma_start(
                            bnc[:, cb + mt * w:cb + (mt + 1) * w],
                            vst[:, c0:c1])

            def k_tile_store(i, bnc, col0, w_tiles=None):
                kst = st1_pool.tile([128, S], bf16, name="kst",
                                    tag="kst", bufs=3)
                kq_tile(w_tiles or wqK, i, kst)
                nc.sync.dma_start(bnc[:, col0:col0 + S], kst[:])

            # pair-0's K tile (from the prioritized weight slice), then the
            # V first half distributed into the pair-0/1/2 buffers, then one
            # gather per pair in consumption order
            k_tile_store(0, bnc_g[0], 0, wqK0)
            v_tiles([(bnc_g[0], 512, 0, 128), (bnc_g[1], 512, 128, 256),
                     (bnc_g[2], 512, 256, 384)], 0)
            allgather(bnc_g[0], gat_g[0])
            k_tile_store(1, bnc_g[1], 0)
            allgather(bnc_g[1], gat_g[1])
            k_tile_store(2, bnc_g[2], 0)
            allgather(bnc_g[2], gat_g[2])
            # Q tiles 0-2 (needed by first pairs)
            for i in range(3):
                kq_tile(wqQ, i, qT_sb[i])
            # second half: K3 + V heads 6-11, then per-pair gathers
            k_tile_store(3, bnc_g[3], 0)
            v_tiles([(bnc_g[3], 512, 0, 128), (bnc_g[4], 512, 128, 256),
                     (bnc_g[5], 512, 256, 384)], C // 2)
            allgather(bnc_g[3], gat_g[3])
            k_tile_store(4, bnc_g[4], 0)
            allgather(bnc_g[4], gat_g[4])
            k_tile_store(5, bnc_g[5], 0)
            allgather(bnc_g[5], gat_g[5])
            # Q tiles 3-5
            for i in range(3, CT):
                kq_tile(wqQ, i, qT_sb[i])

            # w_proj load + bf16 conversion on Pool (idle engine)
            wp_sb = [xw_pool.tile([128, C], f32, name=f"wp{k}")
                     for k in range(CT)]
            for k in range(CT):
                nc.sync.dma_start(wp_sb[k][:],
                                  w_proj[128 * k:128 * (k + 1), :])
            for k in range(CT):
                nc.gpsimd.tensor_copy(wpb_sb[k][:], wp_sb[k][:])
            nc.gpsimd.tensor_copy(bpb_sb[:], bp_sb[:])

        # ---- phase 2: attention ----
        # Single PE tiling mode (64x128) throughout: score MMs contract
        # D=64 per head on T0/T8; chain MMs contract keys split lo/hi on
        # T0/T8 into separate PSUM banks (ob_lo / ob_hi), summed at
        # finalize. Z rides along as the ones-row 64 of both chains.
        with tc.tile_pool(name="kt", bufs=1) as kt_pool, \
             tc.tile_pool(name="vt", bufs=1) as vt_pool, \
             tc.tile_pool(name="pt", bufs=1) as pt_pool, \
             tc.tile_pool(name="fin", bufs=1) as fin_pool, \
             tc.tile_pool(name="sc", bufs=1, space="PSUM") as sc_pool, \
             tc.tile_pool(name="ob", bufs=1, space="PSUM") as ob_pool:

            def finalize_pair(hp, ob_lo, ob_hi):
                # ACT (only engine with spare PSUM-read capacity) drains
                # both chain halves to SBUF fast, freeing the ob banks for
                # the next pair's chains; Pool does the adds/normalize
                # multiply; DVE only the reciprocal.
                for sub in range(2):
                    # sub 0 drains on DVE, sub 1 on ACT -> both subs' ob
                    # banks free in parallel for the next pair's chains
                    hi_s = fin_pool.tile([65, S], f32, name="hi_s",
                                         tag=f"hi_s{sub}", bufs=2)
                    lo_s = fin_pool.tile([65, S], f32, name="lo_s",
                                         tag=f"lo_s{sub}", bufs=2)
                    if sub == 0:
                        nc.vector.tensor_copy(hi_s[:], ob_hi[sub][0:65, :])
                        nc.vector.tensor_copy(lo_s[:], ob_lo[sub][0:65, :])
                    else:
                        nc.scalar.activation(hi_s[:], ob_hi[sub][0:65, :],
                                             COPY)
                        nc.scalar.activation(lo_s[:], ob_lo[sub][0:65, :],
                                             COPY)
                    # NOTE: the adds/mul must NOT go on gpsimd — its
                    # tensor_tensor lives in the 'standard' Q7 library while
                    # partition_broadcast lives in 'attn', and mixing them
                    # forces a library reload every pair
                    zt = fin_pool.tile([1, S], f32, name="zt",
                                       tag=f"zt{sub}", bufs=2)
                    nc.vector.tensor_add(zt[:], lo_s[64:65, :],
                                         hi_s[64:65, :])
                    rzv = fin_pool.tile([1, S], f32, name="rzv",
                                        tag=f"rzv{sub}", bufs=2)
                    nc.vector.reciprocal_approx_fast(rzv[:], zt[:])
                    rzb = fin_pool.tile([64, S], f32, name="rzb",
                                        tag=f"rzb{sub}", bufs=2)
                    nc.gpsimd.partition_broadcast(rzb[:], rzv[:])
                    tmp = fin_pool.tile([64, S], f32, name="tmp",
                                        tag=f"tmp{sub}", bufs=2)
                    nc.vector.tensor_add(tmp[:], lo_s[0:64, :],
                                         hi_s[0:64, :])
                    aon = fin_pool.tile([64, S], bf16, name="aon",
                                        tag=f"aon{sub}", bufs=2)
                    nc.vector.tensor_mul(aon[:], rzb[:], tmp[:])
                    nc.sync.dma_start(aoTn[hp][64 * sub:64 * (sub + 1), :],
                                      aon[:])

            for hp in range(HP):
                kt = kt_pool.tile([128, N], bf16, name="kt", tag="kt", bufs=2)
                # two DMAs (first/second core-halves): pair-0's scores for
                # key tiles 0-15 start ~3.5us sooner after the gather lands.
                # The vt loads are issued BETWEEN the halves: chains need vt
                # by t~3-10 while scores need the second kt half only at
                # t=16, so this ordering removes a ~3.5us chain stall.
                src = gat_g[hp].rearrange("(r p) s -> p r s", p=128)
                kt_v = kt[:].rearrange("p (r s) -> p r s", r=CORES)
                nc.sync.dma_start(kt_v[:, 0:CORES // 2, :],
                                  src[:, 0:CORES // 2, 0:S])

                vts = []
                for sub in range(2):
                    vt = vt_pool.tile([128, KT * 65], bf16, name=f"vt{sub}",
                                      tag=f"vt{sub}", bufs=2)
                    vt_v = vt[:].rearrange("p (t c) -> p t c", c=65)
                    # DVE, not Pool: Pool's FIFO carries the previous
                    # pair's finalize arithmetic and would delay this
                    nc.vector.memset(vt_v[:, :, D], 1.0)
                    # 4 DMAs (one per mt): vt[p, 4r+mt, 0:D] <-
                    # gat[128r+p, 512 + mt*128 + sub*64 : +D]
                    vt_v4 = vt[:].rearrange("p (r t c) -> p r t c",
                                            r=CORES, c=65)
                    for mt in range(MT):
                        cc0 = 512 + mt * 128 + sub * D
                        nc.sync.dma_start(vt_v4[:, :, mt, 0:D],
                                          src[:, :, cc0:cc0 + D])
                    vts.append(vt)
                nc.sync.dma_start(kt_v[:, CORES // 2:CORES, :],
                                  src[:, CORES // 2:CORES, 0:S])

                ob_lo = [ob_pool.tile([128, S], f32, name=f"obl{sub}",
                                      tag=f"obl{sub}", bufs=1)
                         for sub in range(2)]
                ob_hi = [ob_pool.tile([128, S], f32, name=f"obh{sub}",
                                      tag=f"obh{sub}", bufs=1)
                         for sub in range(2)]

                # chains trail their own pair's scores by a few steps (vt
                # DMA margin; V arrives in the same gather as K)
                # delay 3: at delay 2 the chains catch pt(j) before its
                # exp completes (~2.2us stall per pair in the trace).
                # pair 0 waits its vt DMAs, which now issue between the
                # two kt halves (~7us after the gather sem), so 6 steps
                # at 2/t catch-up suffice.
                delay, rate = (6, 2) if hp == 0 else (3, 1)
                done = [0, 0]

                def chain_step(sub, j):
                    nc.tensor.matmul(
                        ob_lo[sub][0:65, :],
                        vts[sub][0:64, 65 * j:65 * (j + 1)],
                        pts[j][0:64, S * sub:S * (sub + 1)],
                        start=(j == 0), stop=(j == KT - 1))
                    nc.tensor.matmul(
                        ob_hi[sub][0:65, :],
                        vts[sub][64:128, 65 * j:65 * (j + 1)],
                        pts[j][64:128, S * sub:S * (sub + 1)],
                        start=(j == 0), stop=(j == KT - 1))

                def run_chains(t, sub):
                    quota = min(KT, len(pts), max(0, t - delay + 1) * rate)
                    while done[sub] < quota:
                        chain_step(sub, done[sub])
                        done[sub] += 1

                pts = []
                for t in range(KT + 2):
                    if t < KT:
                        sc = sc_pool.tile([128, 2 * S], f32, name="sc",
                                          tag="sc", bufs=2)
                        nc.tensor.matmul(
                            sc[:, 0:S], kt[0:64, 128 * t:128 * (t + 1)],
                            qT_sb[hp][0:64, :], start=True, stop=True)
                        nc.tensor.matmul(
                            sc[:, S:2 * S], kt[64:128, 128 * t:128 * (t + 1)],
                            qT_sb[hp][64:128, :], start=True, stop=True)
                        pt = pt_pool.tile([128, 2 * S], bf16, name="pt",
                                          tag="pt", bufs=12)
                        # DVE pays a pipe-drain ~= op cost on its big exp
                        # ops (effective ~2.2us/tile vs ACT ~1.15us), so
                        # ACT takes 21 of 32 tiles. The last two tiles are
                        # split across both engines: the next pair's first
                        # scores wait on these sc banks, so clearing the
                        # exp backlog fast shrinks the pair-boundary gap.
                        if t >= KT - 2:
                            nc.scalar.activation(pt[:, 0:S], sc[:, 0:S],
                                                 EXP, scale=SCALE)
                            nc.vector.tensor_scalar(
                                pt[:, S:2 * S].bitcast(i16), sc[:, S:2 * S],
                                EXP_A, EXP_MAGIC, MULT, mybir.AluOpType.add)
                        elif (t * 21) % 32 < 21:
                            nc.scalar.activation(pt[:], sc[:], EXP,
                                                 scale=SCALE)
                        else:
                            nc.vector.tensor_scalar(
                                pt[:].bitcast(i16), sc[:], EXP_A, EXP_MAGIC,
                                MULT, mybir.AluOpType.add)
                        pts.append(pt)
                        run_chains(t - 2, 0)
                        run_chains(t - 2, 1)
                    else:
                        run_chains(KT + delay, 0)
                        run_chains(KT + delay, 1)
                finalize_pair(hp, ob_lo, ob_hi)

        # ---- phase 3: output projection + bias ----
        # All 8 (mt, half) groups hold an open accumulation in their own
        # PSUM bank and run k=0..4 first: those 40 MMs overlap the last
        # pair's finalize chain (which produces aoTn[5], the k=5 input).
        # A k=5-early ordering would stall the whole PE FIFO on group 0.
        with tc.tile_pool(name="yst", bufs=1) as y_pool, \
             tc.tile_pool(name="fo", bufs=1, space="PSUM") as fo_pool:
            ysts = [y_pool.tile([128, C], f32, name=f"yst{mt}",
                                tag="yst", bufs=4) for mt in range(MT)]
            halves = ((0, 384), (384, 768))
            fos = {}
            for mt in range(MT):
                for half, (n0, n1) in enumerate(halves):
                    fo = fo_pool.tile([128, 384], f32, name="fo", tag="fo",
                                      bufs=8)
                    for k in range(CT - 1):
                        nc.tensor.matmul(
                            fo[:],
                            aoTn[k][:, 128 * mt:128 * (mt + 1)],
                            wpb_sb[k][:, n0:n1],
                            start=(k == 0), stop=False)
                    fos[(mt, half)] = fo
            for mt in range(MT):
                for half, (n0, n1) in enumerate(halves):
                    fo = fos[(mt, half)]
                    nc.tensor.matmul(
                        fo[:],
                        aoTn[CT - 1][:, 128 * mt:128 * (mt + 1)],
                        wpb_sb[CT - 1][:, n0:n1],
                        start=False, stop=False)
                    nc.tensor.matmul(fo[:], ones_bf[0:1, :],
                                     bpb_sb[0:1, n0:n1],
                                     start=False, stop=True)
                    if (2 * mt + half) % 2 == 0:
                        nc.vector.tensor_copy(ysts[mt][:, n0:n1], fo[:])
                    else:
                        nc.scalar.activation(ysts[mt][:, n0:n1], fo[:], COPY)
                nc.sync.dma_start(y[128 * mt:128 * (mt + 1), :], ysts[mt][:])

    nc.compile()
    return nc


def _get_compiled():
    global _COMPILED
    if _COMPILED is None:
        _COMPILED = _build()
    return _COMPILED


def _run(inputs, trace=False):
    from concourse.bass_utils import run_bass_kernel_spmd

    nc = _get_compiled()
    x = np.asarray(inputs["x"], dtype=np.float32)
    w_qkv = np.ascontiguousarray(np.asarray(inputs["w_qkv"], dtype=np.float32))
    w_proj = np.ascontiguousarray(np.asarray(inputs["w_proj"], dtype=np.float32))
    b_proj = np.ascontiguousarray(
        np.asarray(inputs["b_proj"], dtype=np.float32).reshape(1, C))
    xT_full = np.ascontiguousarray(x[0].T)  # [C, N]

    in_maps = []
    for c in range(CORES):
        in_maps.append({
            "xT": np.ascontiguousarray(xT_full[:, S * c:S * (c + 1)]),
            "w_qkv": w_qkv,
            "w_proj": w_proj,
            "b_proj": b_proj,
        })
    res = run_bass_kernel_spmd(nc, in_maps, core_ids=list(range(CORES)),
                               trace=trace)
    out = np.concatenate([res.results[c]["y"] for c in range(CORES)], axis=0)
    return out[None, :, :].astype(np.float32), res


def _spot_check(out, inputs, rows=(17, 1031, 2063, 3571)) -> bool:
    """Cheap numpy verification of a few output rows (guards against a rare
    intermittent device-side corruption; full fp32 math on 4 rows)."""
    x = np.asarray(inputs["x"], dtype=np.float32)[0]
    w_qkv = np.asarray(inputs["w_qkv"], dtype=np.float32)
    w_proj = np.asarray(inputs["w_proj"], dtype=np.float32)
    b_proj = np.asarray(inputs["b_proj"], dtype=np.float32).reshape(-1)
    kf = x @ w_qkv[:, C:2 * C]
    vf = x @ w_qkv[:, 2 * C:3 * C]
    scale = np.abs(out).max() + 1e-30
    for r in rows:
        q = x[r] @ w_qkv[:, 0:C]
        ao = np.empty(C, np.float32)
        for h in range(H):
            sc = kf[:, D * h:D * (h + 1)] @ q[D * h:D * (h + 1)] * SCALE
            p = np.exp(sc - sc.max())
            ao[D * h:D * (h + 1)] = (p @ vf[:, D * h:D * (h + 1)]) / p.sum()
        yr = ao @ w_proj + b_proj
        if np.abs(out[0, r] - yr).max() / scale > 5e-2:
            return False
    return True


def kernel(**inputs) -> np.ndarray:
    out, _ = _run(inputs, trace=False)
    for _retry in range(2):
        if _spot_check(out, inputs):
            break
        out, _ = _run(inputs, trace=False)
    return out
